# revision 16
# baseline (speedup 1.0000x reference)
"""Gaussian-noise kernel for Trainium2: out = clip(x + noise, 0, 1).

Full input shape (64, 3, 512, 512) f32; pure data-parallel over the batch
dim across 8 NeuronCores (8 images per core). Per core the work is a flat
elementwise pass over 6,291,456 floats: DMA x and noise tiles into SBUF,
add on the vector engine, clip with one dual-op tensor_scalar (max 0,
min 1) casting to bf16, DMA the bf16 result back out (upcast to f32 on
the host). Storing bf16 keeps max rel err at 2^-8 (~4e-3, well inside
the 2e-2 gate — bf16 keeps f32's exponent range so small outputs stay
accurate) and cuts HBM traffic from 75.5 MB to 62.9 MB per core per
pass; the kernel is HBM-bandwidth-bound (~358 GB/s/core cap).

The per-core flat buffer is viewed as [N_CHUNKS, 128, CHUNK] so each
chunk's DMA is one fully contiguous block of DRAM.
"""

import numpy as np

import concourse.bacc as bacc
import concourse.bass as bass
import concourse.mybir as mybir
from concourse.bass_utils import run_bass_kernel_spmd
from concourse.tile import TileContext

N_CORES = 8
B, C, H, W = 64, 3, 512, 512
PER_CORE_ELEMS = (B // N_CORES) * C * H * W  # 6,291,456
P = 128
FREE = PER_CORE_ELEMS // P  # 49,152

# tuned knobs (hardware-swept: chunk/bufs/ring assignment all within ~2% of
# each other; c2048_b8 + alternating store rings was best-or-tied in every
# clean measurement window)
CHUNK = 2048
BUFS = 8
CONTIG = True          # view DRAM as [n_chunks, P, CHUNK] (contiguous chunks)
STORE_SCALAR = False   # (unused when STORE_ALT: store ring alternates)
SPLIT_LOADS = True     # x loads on SP ring, noise loads on ACT ring
STORE_GPSIMD = False   # issue store DMAs via SWDGE (gpsimd) instead
STORE_ALT = True       # alternate store DMAs between the SP and ACT rings
OUT_BF16 = True        # store the clipped result as bf16 (rel err <= 2^-9,
                       # well inside the 2e-2 gate; cuts store HBM traffic 2x)

_cached_nc = None


def _build(repeat: int = 1, chunk: int = CHUNK, bufs: int = BUFS,
           contig: bool = CONTIG, store_scalar: bool = STORE_SCALAR,
           split_loads: bool = SPLIT_LOADS, store_gpsimd: bool = STORE_GPSIMD,
           store_alt: bool = STORE_ALT, loads_alt: bool = False,
           taper: bool = False, out_bf16: bool = OUT_BF16,
           store_split: bool = False, diag_loads_only: bool = False):
    n_chunks = FREE // chunk
    assert n_chunks * chunk == FREE

    nc = bacc.Bacc("TRN2", target_bir_lowering=False, debug=False)
    f32 = mybir.dt.float32
    out_dt = mybir.dt.bfloat16 if out_bf16 else f32
    if contig:
        shape = (n_chunks, P, chunk)
    else:
        shape = (P, FREE)
    x = nc.dram_tensor("x", shape, f32, kind="ExternalInput").ap()
    noise = nc.dram_tensor("noise", shape, f32, kind="ExternalInput").ap()
    out = nc.dram_tensor("out", shape, out_dt, kind="ExternalOutput").ap()

    def chunk_ap(ap, i):
        if contig:
            return ap[i]
        return ap[:, bass.ts(i, chunk)]

    store_eng_load = nc.scalar if split_loads else nc.sync
    store_eng = nc.gpsimd if store_gpsimd else (nc.scalar if store_scalar else nc.sync)

    with TileContext(nc) as tc:
        with tc.tile_pool(name="io", bufs=bufs) as pool:

            def emit(i, lo, width):
                """One pipelined unit covering chunk i's [lo, lo+width) slice."""
                xt = pool.tile([P, width], f32, tag="x")
                nt = pool.tile([P, width], f32, tag="n")
                if out_bf16:
                    ot = pool.tile([P, width], out_dt, tag="o")
                else:
                    ot = xt
                if loads_alt:
                    x_eng = nc.sync if i % 2 == 0 else nc.scalar
                    n_eng = nc.scalar if i % 2 == 0 else nc.sync
                else:
                    x_eng, n_eng = nc.sync, store_eng_load
                sub = (lambda ap: ap if width == chunk
                       else ap[:, lo:lo + width])
                x_eng.dma_start(out=xt, in_=sub(chunk_ap(x, i)))
                n_eng.dma_start(out=nt, in_=sub(chunk_ap(noise, i)))
                if diag_loads_only:
                    # bench-only: measure the pure-read ceiling; output is
                    # garbage (only chunk 0 gets stored, computed from loads)
                    if i != 0:
                        return
                nc.vector.tensor_add(out=xt, in0=xt, in1=nt)
                # clip in f32, cast to the store dtype on the way out
                nc.vector.tensor_scalar(
                    out=ot,
                    in0=xt,
                    scalar1=0.0,
                    scalar2=1.0,
                    op0=mybir.AluOpType.max,
                    op1=mybir.AluOpType.min,
                )
                if store_split:
                    half = width // 2
                    dst = sub(chunk_ap(out, i))
                    nc.sync.dma_start(out=dst[:, :half], in_=ot[:, :half])
                    nc.scalar.dma_start(out=dst[:, half:], in_=ot[:, half:])
                else:
                    s_eng = (nc.sync if i % 2 == 1 else nc.scalar) if store_alt else store_eng
                    s_eng.dma_start(out=sub(chunk_ap(out, i)), in_=ot)

            def body():
                for i in range(n_chunks):
                    if taper and i in (0, n_chunks - 1):
                        half = chunk // 2
                        emit(i, 0, half)
                        emit(i, half, half)
                    else:
                        emit(i, 0, chunk)

            if repeat == 1:
                body()
            else:
                with tc.For_i(0, repeat, 1):
                    body()
    nc.compile()
    return nc


def _get_nc():
    global _cached_nc
    if _cached_nc is None:
        _cached_nc = _build()
    return _cached_nc


def _shard(a: np.ndarray, contig: bool = CONTIG, chunk: int = CHUNK):
    n_chunks = FREE // chunk
    a = np.ascontiguousarray(a, dtype=np.float32)
    if contig:
        return a.reshape(N_CORES, n_chunks, P, chunk)
    return a.reshape(N_CORES, P, FREE)


# Cached PJRT executor: trace/compile the sharded bass_exec once per process
# so repeat kernel() calls only pay data transfer + execution.
_cached_fn = None


def _make_fn(nc):
    """Compile `nc` into an 8-core shard_map callable.

    Returns (fn, in_names, sharding, zeros_global): call
    fn(*per_core_inputs_in_in_names_order, *zeros_global) -> (out,).
    """
    import jax
    from jax.sharding import Mesh, NamedSharding, PartitionSpec
    from jax.experimental.shard_map import shard_map
    from concourse.bass2jax import (
        _bass_exec_p,
        install_neuronx_cc_hook,
        partition_id_tensor,
    )

    install_neuronx_cc_hook()
    partition_name = nc.partition_id_tensor.name if nc.partition_id_tensor else None

    in_names, out_names, out_avals, zero_outs = [], [], [], []
    for alloc in nc.m.functions[0].allocations:
        if not isinstance(alloc, mybir.MemoryLocationSet):
            continue
        name = alloc.memorylocations[0].name
        if alloc.kind == "ExternalInput":
            if name != partition_name:
                in_names.append(name)
        elif alloc.kind == "ExternalOutput":
            out_names.append(name)
            shape = tuple(alloc.tensor_shape)
            dtype = mybir.dt.np(alloc.dtype)
            out_avals.append(jax.core.ShapedArray(shape, dtype))
            zero_outs.append(np.zeros(shape, dtype))
    n_params = len(in_names)
    all_in_names = list(in_names) + list(out_names)
    if partition_name is not None:
        all_in_names.append(partition_name)

    def _body(*args):
        operands = list(args)
        if partition_name is not None:
            operands.append(partition_id_tensor())
        outs = _bass_exec_p.bind(
            *operands,
            out_avals=tuple(out_avals),
            in_names=tuple(all_in_names),
            out_names=tuple(out_names),
            lowering_input_output_aliases=(),
            sim_require_finite=True,
            sim_require_nnan=True,
            nc=nc,
        )
        return tuple(outs)

    devices = jax.devices()[:N_CORES]
    mesh = Mesh(np.asarray(devices), ("core",))
    in_specs = (PartitionSpec("core"),) * (n_params + len(out_names))
    out_specs = (PartitionSpec("core"),) * len(out_names)
    fn = jax.jit(
        shard_map(_body, mesh=mesh, in_specs=in_specs, out_specs=out_specs,
                  check_rep=False),
        keep_unused=True,
    )
    sharding = NamedSharding(mesh, PartitionSpec("core"))
    zeros_global = [np.concatenate([z] * N_CORES, axis=0) for z in zero_outs]
    return (fn, in_names, sharding, zeros_global)


def _get_fn():
    global _cached_fn
    if _cached_fn is None:
        _cached_fn = _make_fn(_get_nc())
    return _cached_fn


def _kernel_fast(x: np.ndarray, noise: np.ndarray) -> np.ndarray:
    import jax

    fn, in_names, sharding, zeros_global = _get_fn()
    per_core = {"x": _shard(x), "noise": _shard(noise)}
    args = []
    for name in in_names:
        a = per_core[name]
        args.append(jax.device_put(
            np.ascontiguousarray(a.reshape(-1, *a.shape[2:])), sharding))
    for z in zeros_global:
        args.append(jax.device_put(z, sharding))
    out = fn(*args)[0]
    return np.asarray(out).reshape(B, C, H, W).astype(np.float32, copy=False)


def _kernel_stock(x: np.ndarray, noise: np.ndarray) -> np.ndarray:
    nc = _get_nc()
    xs = _shard(x)
    ns = _shard(noise)
    in_maps = [{"x": xs[c], "noise": ns[c]} for c in range(N_CORES)]
    res = run_bass_kernel_spmd(nc, in_maps, core_ids=list(range(N_CORES)))
    out = np.stack([res.results[c]["out"] for c in range(N_CORES)])
    return out.reshape(B, C, H, W).astype(np.float32, copy=False)


_fast_broken = False
_warmed_up = False


def _device_warmup():
    """Absorb the transient device-unrecoverable flake some fresh processes
    hit on their first NEFF execution (teardown race with a prior process)."""
    global _warmed_up
    if _warmed_up:
        return
    import time
    import jax
    for attempt in range(3):
        try:
            jax.block_until_ready(
                jax.device_put(np.zeros(8, np.float32), jax.devices()[0]) + 1)
            _warmed_up = True
            return
        except Exception:
            time.sleep(10 * (attempt + 1))


def kernel(x: np.ndarray, noise: np.ndarray) -> np.ndarray:
    global _fast_broken, _cached_fn
    import time
    import traceback
    for attempt in range(3):
        if _fast_broken:
            break
        try:
            _device_warmup()
            return _kernel_fast(x, noise)
        except Exception:
            traceback.print_exc()
            if attempt == 2:
                _fast_broken = True
                break
            # reset the jax client in case the PJRT mesh is desynced
            _cached_fn = None
            try:
                import jax
                jax.clear_caches()
                if hasattr(jax, "clear_backends"):
                    jax.clear_backends()
            except Exception:
                pass
            time.sleep(15 * (attempt + 1))
    return _kernel_stock(x, noise)

